# revision 4
# baseline (speedup 1.0000x reference)
"""Trainium2 Bass kernel for the stacked per-cell gate computation.

net[b,c,o] = sum_i x[b,i] Wx[c,o,i] + bx[c,o] + sum_h h[b,h] Wh[c,o,h]
cell_input = tanh(net[..., H:]);  input_gate = sigmoid(net[..., :H])

Strategy: concat x,h -> xh [B, 2048]; concat Wx,Wh per cell -> W' [2048 in,
2048 out].  Shard the C=16 cells as 2 per NeuronCore (expert parallel).  Each
core runs a [M=4096 b, K=2048, N=4096 o] matmul with a fused bias-add (DVE) +
sigmoid/tanh (ACT) epilogue, writing bf16.

Precision split: the sigmoid half of each cell's outputs is computed in
fp8-e4m3 with PE DoubleRow double-pumping (2x matmul throughput); the
sigmoid's flat transfer function absorbs the quantization error.  The tanh
half stays in bf16 (its steeper slope would push fp8 error past the accuracy
budget).

Phase schedule: sigmoid phases run one batch-chunk AHEAD of tanh phases
(S0 S1 T0 S2 T1 ... S7 T6 T7) so the bf16 tanh weights (8 MB) are not needed
until ~55us into the kernel, flattening the startup DMA demand curve.
"""

import os
from contextlib import ExitStack

import numpy as np
import ml_dtypes

B = 4096
IN = 1024
H = 1024
C = 16
NCORES = 8
CPC = C // NCORES          # cells per core
K = IN + H                 # contraction dim
KO = K // 128              # k-tiles
OPC = CPC * 2 * H          # output columns per core
NSLAB = OPC // 512         # 512-wide output slabs per core
BCHUNK = 512               # batch rows resident per xh chunk
NMC = B // BCHUNK

# slab n covers output cols [n*512, (n+1)*512); per cell: 2 sigmoid slabs
# then 2 tanh slabs.
SIG_SLABS = [0, 1, 4, 5]
TANH_SLABS = [2, 3, 6, 7]
W8_IDX = {0: 0, 1: 1, 4: 2, 5: 3}
WB_IDX = {2: 0, 3: 1, 6: 2, 7: 3}

SX = 16.0                  # xh fp8 pre-scale (power of 2: exact)
SW = 4096.0                # weight fp8 pre-scale
SCALE = SX * SW            # PSUM carries net*SCALE on sigmoid slabs

BF16 = ml_dtypes.bfloat16
E4M3 = ml_dtypes.float8_e4m3   # IEEE e4m3 (bias 7, max 240) = TRN FP8_EXP4

_CACHE = {}


def _make_tc_class(tile, mybir, ScopedClock):
    """TileContext that never emits more than one sem-wait per instruction
    (this walrus build rejects multi-wait instructions in codegen)."""

    class SplitWaitTC(tile.TileContext):
        MAXW = 1

        def _split_waits(self, inst):
            si = getattr(inst, "sync_info", None)
            if si is None or len(si.on_wait) <= self.MAXW:
                return None
            waits = list(si.on_wait)
            inst.sync_info = mybir.SyncInfo(
                on_wait=waits[: self.MAXW], on_update=list(si.on_update)
            )
            nops = []
            for i in range(self.MAXW, len(waits), self.MAXW):
                nops.append(
                    mybir.InstNoOp(
                        name=self.nc.get_next_instruction_name(),
                        engine=inst.engine,
                        bass_nofuse=True,
                        sync_info=mybir.SyncInfo(
                            on_wait=waits[i : i + self.MAXW], on_update=[]
                        ),
                    )
                )
            return nops

        def _commit_and_lower(self, inst, original_block, old_bb_map, bb_to_exit_bb):
            nops = self._split_waits(inst)
            if nops:
                for nop in nops:
                    self._commit_instruction(nop)
            return super()._commit_and_lower(
                inst, original_block, old_bb_map, bb_to_exit_bb
            )

        def _drain_and_barrier(self, tick_clock, wait_clock):
            nc = self.nc
            drain_inst = nc.sync.drain()
            wait_clock.add_sem_waits(
                drain_inst.ins, ScopedClock({None: tick_clock.global_clock})
            )
            # Hoisting surplus waits onto trailing SP nops keeps semantics:
            # SP is FIFO, and the barrier below only passes once SP has
            # cleared every wait.
            si = drain_inst.ins.sync_info
            if si is not None and len(si.on_wait) > self.MAXW:
                waits = list(si.on_wait)
                drain_inst.ins.sync_info = mybir.SyncInfo(
                    on_wait=waits[: self.MAXW], on_update=list(si.on_update)
                )
                for i in range(self.MAXW, len(waits), self.MAXW):
                    nop = nc.sync.nop(nofuse=True)
                    nop.ins.sync_info = mybir.SyncInfo(
                        on_wait=waits[i : i + self.MAXW], on_update=[]
                    )
            nc.all_engine_barrier()
            assert self.sems is not None
            popped = nc._tile_sem_poison_stack.pop()
            assert popped is self._sem_poison
            nc.clear_and_free_semaphores(list(self.sems.allocated().values()))
            nc.all_engine_barrier()

    return SplitWaitTC


def _build():
    import concourse.bass as bass
    import concourse.tile as tile
    from concourse import mybir
    from concourse.vector_clock import ScopedClock

    SplitWaitTC = _make_tc_class(tile, mybir, ScopedClock)

    f32 = mybir.dt.float32
    bf16 = mybir.dt.bfloat16
    fp8 = mybir.dt.float8e4
    AF = mybir.ActivationFunctionType
    DR = mybir.MatmulPerfMode.DoubleRow

    nc = bass.Bass("TRN2", target_bir_lowering=False, debug=False)
    xh8_ap = nc.dram_tensor(
        "xh8", [NMC, 128, KO, BCHUNK], fp8, kind="ExternalInput"
    ).ap()
    xhb_ap = nc.dram_tensor(
        "xhb", [NMC, 128, KO, BCHUNK], bf16, kind="ExternalInput"
    ).ap()
    # First m-tile's lhsT duplicated in a tiny tensor so PE can start early.
    xh00_ap = nc.dram_tensor(
        "xh00", [128, KO, 128], fp8, kind="ExternalInput"
    ).ap()
    w8_ap = nc.dram_tensor(
        "w8", [len(SIG_SLABS), 128, KO, 512], fp8, kind="ExternalInput"
    ).ap()
    wb_ap = nc.dram_tensor(
        "wb", [len(TANH_SLABS), 128, KO, 512], bf16, kind="ExternalInput"
    ).ap()
    bias_ap = nc.dram_tensor("bias", [128, OPC], bf16, kind="ExternalInput").ap()
    out_ap = nc.dram_tensor("out", [B, OPC], bf16, kind="ExternalOutput").ap()

    with SplitWaitTC(nc) as tc:
        with ExitStack() as ctx:
            wpool = ctx.enter_context(tc.tile_pool(name="w", bufs=1))
            x8pool = ctx.enter_context(tc.tile_pool(name="xh8", bufs=2))
            xbpool = ctx.enter_context(tc.tile_pool(name="xhb", bufs=2))
            bpool = ctx.enter_context(tc.tile_pool(name="bias", bufs=1))
            pspool = ctx.enter_context(tc.tile_pool(name="ps", bufs=8, space="PSUM"))
            tpool = ctx.enter_context(tc.tile_pool(name="tmp", bufs=3))
            opool = ctx.enter_context(tc.tile_pool(name="o", bufs=3))

            w8_t = {}
            for n in SIG_SLABS:
                w8_t[n] = wpool.tile(
                    [128, KO, 512], fp8, tag=f"w8_{n}", name=f"w8_{n}"
                )
            wb_t = {}
            for n in TANH_SLABS:
                wb_t[n] = wpool.tile(
                    [128, KO, 512], bf16, tag=f"wb_{n}", name=f"wb_{n}"
                )
            xh8_sb = {}
            xhb_sb = {}
            xh8_sb[0] = x8pool.tile([128, KO, BCHUNK], fp8, tag="xh8", name="xh8_c0")
            xh8_sb[1] = x8pool.tile([128, KO, BCHUNK], fp8, tag="xh8", name="xh8_c1")
            xhb_sb[0] = xbpool.tile([128, KO, BCHUNK], bf16, tag="xhb", name="xhb_c0")
            xh00 = bpool.tile([128, KO, 128], fp8, tag="xh00", name="xh00")
            bias_sb = bpool.tile([128, OPC], bf16)

            # Startup loads in strict first-needed order per queue.  The
            # sync queue carries the critical path (xh00 gates the first
            # matmul; w8 slabs feed the opening sigmoid phases).  bf16 tanh
            # data is not needed until ~55us (phase T0), so it rides on the
            # scalar/gpsimd queues.
            HK = KO // 2
            nc.sync.dma_start(xh00[:], xh00_ap[:])
            nc.sync.dma_start(w8_t[0][:, :HK, :], w8_ap[0, :, :HK, :])
            nc.sync.dma_start(w8_t[0][:, HK:, :], w8_ap[0, :, HK:, :])
            nc.sync.dma_start(bias_sb[:, : OPC // 2], bias_ap[:, : OPC // 2])
            nc.sync.dma_start(xh8_sb[0][:, :HK, :], xh8_ap[0, :, :HK, :])
            nc.sync.dma_start(xh8_sb[0][:, HK:, :], xh8_ap[0, :, HK:, :])
            nc.sync.dma_start(w8_t[1][:], w8_ap[1, :, :, :])
            nc.sync.dma_start(bias_sb[:, OPC // 2 :], bias_ap[:, OPC // 2 :])
            nc.sync.dma_start(w8_t[4][:], w8_ap[2, :, :, :])
            nc.sync.dma_start(w8_t[5][:], w8_ap[3, :, :, :])
            nc.scalar.dma_start(xhb_sb[0][:], xhb_ap[0, :, :, :])
            nc.scalar.dma_start(wb_t[2][:], wb_ap[0, :, :, :])
            nc.scalar.dma_start(wb_t[3][:], wb_ap[1, :, :, :])
            nc.gpsimd.dma_start(xh8_sb[1][:], xh8_ap[1, :, :, :])
            nc.gpsimd.dma_start(wb_t[6][:], wb_ap[2, :, :, :])
            nc.gpsimd.dma_start(wb_t[7][:], wb_ap[3, :, :, :])

            def do_tile(mc, n, mi, nsplit, is_sig):
                func = AF.Sigmoid if is_sig else AF.Tanh
                width = 512 // nsplit
                row0 = mc * BCHUNK + mi * 128
                for sp in range(nsplit):
                    c0 = sp * width
                    ps = pspool.tile(
                        [128, width],
                        mybir.dt.float32,
                        tag="ps",
                        name=f"ps_{mc}_{n}_{mi}_{sp}",
                    )
                    if is_sig:
                        for k in range(KO // 2):
                            if mc == 0 and mi == 0:
                                lhsT = xh00[:, 2 * k : 2 * k + 2, :]
                            else:
                                lhsT = xh8_sb[mc][
                                    :, 2 * k : 2 * k + 2, mi * 128 : (mi + 1) * 128
                                ]
                            nc.tensor.matmul(
                                ps[:],
                                lhsT,
                                w8_t[n][:, 2 * k : 2 * k + 2, c0 : c0 + width],
                                start=(k == 0),
                                stop=(k == KO // 2 - 1),
                                perf_mode=DR,
                            )
                    else:
                        for k in range(KO):
                            nc.tensor.matmul(
                                ps[:],
                                xhb_sb[mc][:, k, mi * 128 : (mi + 1) * 128],
                                wb_t[n][:, k, c0 : c0 + width],
                                start=(k == 0),
                                stop=(k == KO - 1),
                            )
                    tmp = tpool.tile([128, width], mybir.dt.float32, tag="tmp")
                    nc.vector.tensor_tensor(
                        tmp[:],
                        ps[:],
                        bias_sb[:, n * 512 + c0 : n * 512 + c0 + width],
                        mybir.AluOpType.add,
                    )
                    o_t = opool.tile([128, width], bf16, tag="o")
                    if is_sig:
                        nc.scalar.activation(o_t[:], tmp[:], func, scale=1.0 / SCALE)
                    else:
                        nc.scalar.activation(o_t[:], tmp[:], func)
                    nc.sync.dma_start(
                        out_ap[
                            row0 : row0 + 128,
                            n * 512 + c0 : n * 512 + c0 + width,
                        ],
                        o_t[:],
                    )

            # Phase sequence: S0 S1 T0 S2 T1 ... S7 T6 T7.
            phases = [("S", 0)]
            for k in range(1, NMC):
                phases.append(("S", k))
                phases.append(("T", k - 1))
            phases.append(("T", NMC - 1))

            for kind, mc in phases:
                if kind == "S":
                    if mc >= 1:
                        # prefetch next chunks (~2 phases of lead time)
                        xhb_sb[mc] = xbpool.tile(
                            [128, KO, BCHUNK], bf16, tag="xhb", name=f"xhb_c{mc}"
                        )
                        nc.sync.dma_start(xhb_sb[mc][:], xhb_ap[mc, :, :, :])
                        if mc + 1 < NMC and mc + 1 >= 2:
                            xh8_sb[mc + 1] = x8pool.tile(
                                [128, KO, BCHUNK], fp8, tag="xh8",
                                name=f"xh8_c{mc + 1}",
                            )
                            nc.gpsimd.dma_start(
                                xh8_sb[mc + 1][:], xh8_ap[mc + 1, :, :, :]
                            )
                    for n in SIG_SLABS:
                        for mi in range(BCHUNK // 128):
                            do_tile(mc, n, mi, 1, True)
                else:
                    last_phase = mc == NMC - 1
                    for n in TANH_SLABS:
                        for mi in range(BCHUNK // 128):
                            nsplit = (
                                4
                                if (last_phase and n == TANH_SLABS[-1]
                                    and mi == BCHUNK // 128 - 1)
                                else 1
                            )
                            do_tile(mc, n, mi, nsplit, False)
    return nc


def _q8(arr, scale):
    return np.clip(arr * scale, -240.0, 240.0).astype(E4M3)


def _install_ntff_hook():
    """Recreate the missing antenv.axon_hooks module so trace=True works."""
    import sys, types, ctypes, contextlib

    if "antenv.axon_hooks" in sys.modules:
        return
    so_path = "/opt/axon/libaxon_pjrt.so"
    lib = ctypes.CDLL(so_path)
    if not hasattr(lib, "axon_start_nrt_profile"):
        return
    lib.axon_start_nrt_profile.argtypes = [
        ctypes.POINTER(ctypes.c_int64),
        ctypes.c_size_t,
    ]
    lib.axon_start_nrt_profile.restype = ctypes.c_int64
    lib.axon_stop_nrt_profile.argtypes = [ctypes.c_char_p]
    lib.axon_stop_nrt_profile.restype = ctypes.c_int64

    @contextlib.contextmanager
    def _hook(output_dir, device_ids):
        import jax

        jax.devices()
        if device_ids:
            ids = (ctypes.c_int64 * len(device_ids))(*device_ids)
            rc = lib.axon_start_nrt_profile(ids, len(device_ids))
        else:
            rc = lib.axon_start_nrt_profile(None, 0)
        if rc != 0:
            raise RuntimeError(f"axon_start_nrt_profile rc={rc}")
        try:
            yield
        finally:
            n = lib.axon_stop_nrt_profile(str(output_dir).encode())
            if n < 0:
                raise RuntimeError(f"axon_stop_nrt_profile rc={n}")
            print(f"profile: {n} file(s) written to {output_dir}")

    mod = types.ModuleType("antenv.axon_hooks")
    mod.get_axon_ntff_profile_hook = lambda: _hook
    mod.set_axon_ntff_profile_hook = lambda h: None
    sys.modules["antenv.axon_hooks"] = mod


def kernel(input_word, hidden_states, Wx, bx, Wh):
    from concourse import bass_utils

    x = np.asarray(input_word, dtype=np.float32)
    h = np.asarray(hidden_states, dtype=np.float32)
    Wx = np.asarray(Wx, dtype=np.float32)
    bx = np.asarray(bx, dtype=np.float32)
    Wh = np.asarray(Wh, dtype=np.float32)

    xh = np.concatenate([x, h], axis=1)                      # [B, K]
    # [K, B] -> chunk-major [nchunk, 128 p, KO, BCHUNK] with k = ko*128+p.
    xh_sw = np.ascontiguousarray(
        xh.T.reshape(KO, 128, B // BCHUNK, BCHUNK).transpose(2, 1, 0, 3)
    )
    xh8_sw = _q8(xh_sw, SX)
    xhb_sw = xh_sw.astype(BF16)
    xh00 = np.ascontiguousarray(
        xh.T.reshape(KO, 128, B)[:, :, :128].transpose(1, 0, 2)
    )
    xh00_8 = _q8(xh00, SX)

    Wcat = np.concatenate([Wx, Wh], axis=2)                  # [C, 2H, K]
    in_maps = []
    for c0 in range(NCORES):
        wc = np.concatenate(
            [Wcat[CPC * c0 + j].T for j in range(CPC)], axis=1
        )                                                    # [K, OPC]
        w_sl = wc.reshape(KO, 128, NSLAB, 512).transpose(2, 1, 0, 3)
        w8 = _q8(
            np.ascontiguousarray(w_sl[SIG_SLABS]), SW
        )                                                    # [4,128,KO,512]
        wb = np.ascontiguousarray(w_sl[TANH_SLABS]).astype(BF16)
        bias_core = np.concatenate(
            [bx[CPC * c0 + j] for j in range(CPC)]
        ).astype(np.float32)                                 # [OPC]
        # sigmoid slabs carry net*SCALE in PSUM; pre-scale their bias.
        bias_adj = bias_core.copy()
        for n in SIG_SLABS:
            bias_adj[n * 512 : (n + 1) * 512] *= SCALE
        bias_b = np.ascontiguousarray(
            np.broadcast_to(bias_adj.astype(BF16), (128, OPC))
        )
        in_maps.append(
            {
                "xh8": xh8_sw,
                "xhb": xhb_sw,
                "xh00": xh00_8,
                "w8": w8,
                "wb": wb,
                "bias": bias_b,
            }
        )

    if "nc" not in _CACHE:
        _CACHE["nc"] = _build()
    nc = _CACHE["nc"]

    trace = bool(os.environ.get("GATE_TRACE"))
    if trace:
        _install_ntff_hook()
    res = bass_utils.run_bass_kernel_spmd(
        nc, in_maps, core_ids=list(range(NCORES)), trace=trace
    )
    _CACHE["last_result"] = res

    full = np.empty((B, C, 2 * H), np.float32)
    for c0 in range(NCORES):
        o = res.results[c0]["out"].astype(np.float32).reshape(B, CPC, 2 * H)
        for j in range(CPC):
            full[:, CPC * c0 + j, :] = o[:, j, :]
    input_gate = np.ascontiguousarray(full[:, :, :H])
    cell_input = np.ascontiguousarray(full[:, :, H:])
    return (cell_input, input_gate)


# revision 11
# speedup vs baseline: 1.1081x; 1.1081x over previous
"""Trainium2 Bass kernel for the stacked per-cell gate computation.

net[b,c,o] = sum_i x[b,i] Wx[c,o,i] + bx[c,o] + sum_h h[b,h] Wh[c,o,h]
cell_input = tanh(net[..., H:]);  input_gate = sigmoid(net[..., :H])

Strategy: concat x,h -> xh [B, 2048]; concat Wx,Wh per cell -> W' [2048 in,
2048 out].  Shard the C=16 cells as 2 per NeuronCore (expert parallel).  Each
core runs a [M=4096 b, K=2048, N=4096 o] matmul with a fused bias-add (DVE) +
sigmoid/tanh (ACT) epilogue, writing bf16.

Precision split: the sigmoid half of each cell's outputs is computed in
fp8-e4m3 with PE DoubleRow double-pumping (2x matmul throughput); the
sigmoid's flat transfer function absorbs the quantization error.  The tanh
half stays in bf16 (its steeper slope would push fp8 error past the accuracy
budget).

Phase schedule: sigmoid phases run one batch-chunk AHEAD of tanh phases
(S0 S1 T0 S2 T1 ... S7 T6 T7) so the bf16 tanh weights (8 MB) are not needed
until ~55us into the kernel, flattening the startup DMA demand curve.
"""

import os
from contextlib import ExitStack

import numpy as np
import ml_dtypes

B = 4096
IN = 1024
H = 1024
C = 16
NCORES = 8
CPC = C // NCORES          # cells per core
K = IN + H                 # contraction dim
KO = K // 128              # k-tiles
OPC = CPC * 2 * H          # output columns per core
NSLAB = OPC // 512         # 512-wide output slabs per core
BCHUNK = 512               # batch rows resident per xh chunk
NMC = B // BCHUNK

# slab n covers output cols [n*512, (n+1)*512); per cell: 2 sigmoid slabs
# then 2 tanh slabs.
SIG_SLABS = [0, 1, 4, 5]
TANH_SLABS = [2, 3, 6, 7]
W8_IDX = {0: 0, 1: 1, 4: 2, 5: 3}
WB_IDX = {2: 0, 3: 1, 6: 2, 7: 3}

SX = 16.0                  # xh fp8 pre-scale (power of 2: exact)
SW = 4096.0                # weight fp8 pre-scale
SCALE = SX * SW            # PSUM carries net*SCALE on every slab
# Leading k-tiles of the tanh half also run in fp8 DoubleRow; the bf16 tanh
# weights are pre-scaled by SCALE (exact power of 2) so both parts accumulate
# into one PSUM group at a common scale.  KT8=4 keeps rel err ~1.6e-2 (<2e-2).
KT8 = 4                    # tanh k-tiles (of KO=16) computed in fp8

BF16 = ml_dtypes.bfloat16
E4M3 = ml_dtypes.float8_e4m3   # IEEE e4m3 (bias 7, max 240) = TRN FP8_EXP4

_CACHE = {}


def _make_tc_class(tile, mybir, ScopedClock):
    """TileContext that never emits more than one sem-wait per instruction
    (this walrus build rejects multi-wait instructions in codegen)."""

    class SplitWaitTC(tile.TileContext):
        MAXW = 1

        def _split_waits(self, inst):
            si = getattr(inst, "sync_info", None)
            if si is None or len(si.on_wait) <= self.MAXW:
                return None
            waits = list(si.on_wait)
            inst.sync_info = mybir.SyncInfo(
                on_wait=waits[: self.MAXW], on_update=list(si.on_update)
            )
            nops = []
            for i in range(self.MAXW, len(waits), self.MAXW):
                nops.append(
                    mybir.InstNoOp(
                        name=self.nc.get_next_instruction_name(),
                        engine=inst.engine,
                        bass_nofuse=True,
                        sync_info=mybir.SyncInfo(
                            on_wait=waits[i : i + self.MAXW], on_update=[]
                        ),
                    )
                )
            return nops

        def _commit_and_lower(self, inst, original_block, old_bb_map, bb_to_exit_bb):
            nops = self._split_waits(inst)
            if nops:
                for nop in nops:
                    self._commit_instruction(nop)
            return super()._commit_and_lower(
                inst, original_block, old_bb_map, bb_to_exit_bb
            )

        def _drain_and_barrier(self, tick_clock, wait_clock):
            nc = self.nc
            drain_inst = nc.sync.drain()
            wait_clock.add_sem_waits(
                drain_inst.ins, ScopedClock({None: tick_clock.global_clock})
            )
            # Hoisting surplus waits onto trailing SP nops keeps semantics:
            # SP is FIFO, and the barrier below only passes once SP has
            # cleared every wait.
            si = drain_inst.ins.sync_info
            if si is not None and len(si.on_wait) > self.MAXW:
                waits = list(si.on_wait)
                drain_inst.ins.sync_info = mybir.SyncInfo(
                    on_wait=waits[: self.MAXW], on_update=list(si.on_update)
                )
                for i in range(self.MAXW, len(waits), self.MAXW):
                    nop = nc.sync.nop(nofuse=True)
                    nop.ins.sync_info = mybir.SyncInfo(
                        on_wait=waits[i : i + self.MAXW], on_update=[]
                    )
            nc.all_engine_barrier()
            assert self.sems is not None
            popped = nc._tile_sem_poison_stack.pop()
            assert popped is self._sem_poison
            nc.clear_and_free_semaphores(list(self.sems.allocated().values()))
            nc.all_engine_barrier()

    return SplitWaitTC


def _build():
    import concourse.bass as bass
    import concourse.tile as tile
    from concourse import mybir
    from concourse.vector_clock import ScopedClock

    SplitWaitTC = _make_tc_class(tile, mybir, ScopedClock)

    f32 = mybir.dt.float32
    bf16 = mybir.dt.bfloat16
    fp8 = mybir.dt.float8e4
    AF = mybir.ActivationFunctionType
    DR = mybir.MatmulPerfMode.DoubleRow

    nc = bass.Bass("TRN2", target_bir_lowering=False, debug=False)
    xh8_ap = nc.dram_tensor(
        "xh8", [NMC, 128, KO, BCHUNK], fp8, kind="ExternalInput"
    ).ap()
    xhb_ap = nc.dram_tensor(
        "xhb", [NMC, 128, KO - KT8, BCHUNK], bf16, kind="ExternalInput"
    ).ap()
    # First m-tile's lhsT duplicated in a tiny tensor so PE can start early.
    xh00_ap = nc.dram_tensor(
        "xh00", [128, KO, 128], fp8, kind="ExternalInput"
    ).ap()
    w8_ap = nc.dram_tensor(
        "w8", [len(SIG_SLABS), 128, KO, 512], fp8, kind="ExternalInput"
    ).ap()
    wb_ap = nc.dram_tensor(
        "wb", [len(TANH_SLABS), 128, KO - KT8, 512], bf16, kind="ExternalInput"
    ).ap()
    w8t_ap = nc.dram_tensor(
        "w8t", [128, len(TANH_SLABS), KT8, 512], fp8, kind="ExternalInput"
    ).ap()
    bias_ap = nc.dram_tensor("bias", [128, OPC], bf16, kind="ExternalInput").ap()
    out_ap = nc.dram_tensor("out", [B, OPC], bf16, kind="ExternalOutput").ap()

    with SplitWaitTC(nc) as tc:
        with ExitStack() as ctx:
            wpool = ctx.enter_context(tc.tile_pool(name="w", bufs=1))
            x8pool = ctx.enter_context(tc.tile_pool(name="xh8", bufs=3))
            xbpool = ctx.enter_context(tc.tile_pool(name="xhb", bufs=2))
            bpool = ctx.enter_context(tc.tile_pool(name="bias", bufs=1))
            pspool = ctx.enter_context(tc.tile_pool(name="ps", bufs=8, space="PSUM"))
            tpool = ctx.enter_context(tc.tile_pool(name="tmp", bufs=3))
            opool = ctx.enter_context(tc.tile_pool(name="o", bufs=3))

            w8_t = {}
            for n in SIG_SLABS:
                w8_t[n] = wpool.tile(
                    [128, KO, 512], fp8, tag=f"w8_{n}", name=f"w8_{n}"
                )
            wb_t = {}
            for n in TANH_SLABS:
                wb_t[n] = wpool.tile(
                    [128, KO - KT8, 512], bf16, tag=f"wb_{n}", name=f"wb_{n}"
                )
            w8t = wpool.tile(
                [128, len(TANH_SLABS), KT8, 512], fp8, tag="w8t", name="w8t"
            )
            xh8_sb = {}
            xhb_sb = {}
            xh8_sb[0] = x8pool.tile([128, KO, BCHUNK], fp8, tag="xh8", name="xh8_c0")
            xh8_sb[1] = x8pool.tile([128, KO, BCHUNK], fp8, tag="xh8", name="xh8_c1")
            xhb_sb[0] = xbpool.tile(
                [128, KO - KT8, BCHUNK], bf16, tag="xhb", name="xhb_c0"
            )
            xh00 = bpool.tile([128, KO, 128], fp8, tag="xh00", name="xh00")
            bias_sb = bpool.tile([128, OPC], bf16)

            # Startup loads: one full-tensor descriptor each (>=4KB DMA lines
            # per partition; split descriptors tanked the queue to ~40GB/s),
            # in strict first-needed order per queue.  The sync queue carries
            # the critical path (w8 slabs + first fp8 chunk feed the opening
            # sigmoid phases); bf16 tanh data is not needed until ~60us
            # (phase T0), so it rides behind on the scalar/gpsimd queues.
            nc.sync.dma_start(w8_t[0][:], w8_ap[0, :, :, :])
            nc.gpsimd.dma_start(xh00[:], xh00_ap[:])
            nc.sync.dma_start(xh8_sb[0][:], xh8_ap[0, :, :, :])
            nc.sync.dma_start(w8_t[1][:], w8_ap[1, :, :, :])
            nc.sync.dma_start(w8_t[4][:], w8_ap[2, :, :, :])
            nc.sync.dma_start(w8_t[5][:], w8_ap[3, :, :, :])
            nc.scalar.dma_start(bias_sb[:], bias_ap[:])
            nc.scalar.dma_start(xhb_sb[0][:], xhb_ap[0, :, :, :])
            nc.scalar.dma_start(wb_t[2][:], wb_ap[0, :, :, :])
            nc.gpsimd.dma_start(xh8_sb[1][:], xh8_ap[1, :, :, :])
            nc.gpsimd.dma_start(w8t[:], w8t_ap[:])
            nc.gpsimd.dma_start(wb_t[3][:], wb_ap[1, :, :, :])
            nc.gpsimd.dma_start(wb_t[6][:], wb_ap[2, :, :, :])
            nc.gpsimd.dma_start(wb_t[7][:], wb_ap[3, :, :, :])

            def do_tile(mc, n, mi, nsplit, is_sig):
                func = AF.Sigmoid if is_sig else AF.Tanh
                width = 512 // nsplit
                row0 = mc * BCHUNK + mi * 128
                for sp in range(nsplit):
                    c0 = sp * width
                    ps = pspool.tile(
                        [128, width],
                        mybir.dt.float32,
                        tag="ps",
                        name=f"ps_{mc}_{n}_{mi}_{sp}",
                    )
                    if is_sig:
                        for k in range(KO // 2):
                            if mc == 0 and mi == 0:
                                lhsT = xh00[:, 2 * k : 2 * k + 2, :]
                            else:
                                lhsT = xh8_sb[mc][
                                    :, 2 * k : 2 * k + 2, mi * 128 : (mi + 1) * 128
                                ]
                            nc.tensor.matmul(
                                ps[:],
                                lhsT,
                                w8_t[n][:, 2 * k : 2 * k + 2, c0 : c0 + width],
                                start=(k == 0),
                                stop=(k == KO // 2 - 1),
                                perf_mode=DR,
                            )
                    else:
                        si = WB_IDX[n]
                        for k in range(KT8 // 2):
                            nc.tensor.matmul(
                                ps[:],
                                xh8_sb[mc][
                                    :, 2 * k : 2 * k + 2, mi * 128 : (mi + 1) * 128
                                ],
                                w8t[:, si, 2 * k : 2 * k + 2, c0 : c0 + width],
                                start=(k == 0),
                                stop=False,
                                perf_mode=DR,
                            )
                        for k in range(KO - KT8):
                            nc.tensor.matmul(
                                ps[:],
                                xhb_sb[mc][:, k, mi * 128 : (mi + 1) * 128],
                                wb_t[n][:, k, c0 : c0 + width],
                                start=False,
                                stop=(k == KO - KT8 - 1),
                            )
                    tmp = tpool.tile([128, width], mybir.dt.float32, tag="tmp")
                    nc.vector.tensor_tensor(
                        tmp[:],
                        ps[:],
                        bias_sb[:, n * 512 + c0 : n * 512 + c0 + width],
                        mybir.AluOpType.add,
                    )
                    o_t = opool.tile([128, width], bf16, tag="o")
                    nc.scalar.activation(o_t[:], tmp[:], func, scale=1.0 / SCALE)
                    out_q = (nc.sync, nc.scalar, nc.gpsimd)[sp % 3] if nsplit > 1 else nc.sync
                    out_q.dma_start(
                        out_ap[
                            row0 : row0 + 128,
                            n * 512 + c0 : n * 512 + c0 + width,
                        ],
                        o_t[:],
                    )

            # Phase sequence: S0 S1 T0 S2 T1 ... S7 T6 T7.
            phases = [("S", 0)]
            for k in range(1, NMC):
                phases.append(("S", k))
                phases.append(("T", k - 1))
            phases.append(("T", NMC - 1))

            for kind, mc in phases:
                if kind == "S":
                    if mc >= 1:
                        # prefetch next chunks (~2 phases of lead time)
                        xhb_sb[mc] = xbpool.tile(
                            [128, KO - KT8, BCHUNK], bf16, tag="xhb",
                            name=f"xhb_c{mc}",
                        )
                        nc.sync.dma_start(xhb_sb[mc][:], xhb_ap[mc, :, :, :])
                        if mc + 1 < NMC and mc + 1 >= 2:
                            xh8_sb[mc + 1] = x8pool.tile(
                                [128, KO, BCHUNK], fp8, tag="xh8",
                                name=f"xh8_c{mc + 1}",
                            )
                            nc.gpsimd.dma_start(
                                xh8_sb[mc + 1][:], xh8_ap[mc + 1, :, :, :]
                            )
                    for n in SIG_SLABS:
                        for mi in range(BCHUNK // 128):
                            do_tile(mc, n, mi, 1, True)
                else:
                    last_phase = mc == NMC - 1
                    for n in TANH_SLABS:
                        for mi in range(BCHUNK // 128):
                            nsplit = (
                                4
                                if (last_phase and n == TANH_SLABS[-1]
                                    and mi == BCHUNK // 128 - 1)
                                else 1
                            )
                            do_tile(mc, n, mi, nsplit, False)
    return nc


def _q8(arr, scale):
    return np.clip(arr * scale, -240.0, 240.0).astype(E4M3)


def _install_ntff_hook():
    """Recreate the missing antenv.axon_hooks module so trace=True works."""
    import sys, types, ctypes, contextlib

    if "antenv.axon_hooks" in sys.modules:
        return
    so_path = "/opt/axon/libaxon_pjrt.so"
    lib = ctypes.CDLL(so_path)
    if not hasattr(lib, "axon_start_nrt_profile"):
        return
    lib.axon_start_nrt_profile.argtypes = [
        ctypes.POINTER(ctypes.c_int64),
        ctypes.c_size_t,
    ]
    lib.axon_start_nrt_profile.restype = ctypes.c_int64
    lib.axon_stop_nrt_profile.argtypes = [ctypes.c_char_p]
    lib.axon_stop_nrt_profile.restype = ctypes.c_int64

    @contextlib.contextmanager
    def _hook(output_dir, device_ids):
        import jax

        jax.devices()
        if device_ids:
            ids = (ctypes.c_int64 * len(device_ids))(*device_ids)
            rc = lib.axon_start_nrt_profile(ids, len(device_ids))
        else:
            rc = lib.axon_start_nrt_profile(None, 0)
        if rc != 0:
            raise RuntimeError(f"axon_start_nrt_profile rc={rc}")
        try:
            yield
        finally:
            n = lib.axon_stop_nrt_profile(str(output_dir).encode())
            if n < 0:
                raise RuntimeError(f"axon_stop_nrt_profile rc={n}")
            print(f"profile: {n} file(s) written to {output_dir}")

    mod = types.ModuleType("antenv.axon_hooks")
    mod.get_axon_ntff_profile_hook = lambda: _hook
    mod.set_axon_ntff_profile_hook = lambda h: None
    sys.modules["antenv.axon_hooks"] = mod


def kernel(input_word, hidden_states, Wx, bx, Wh):
    from concourse import bass_utils

    x = np.asarray(input_word, dtype=np.float32)
    h = np.asarray(hidden_states, dtype=np.float32)
    Wx = np.asarray(Wx, dtype=np.float32)
    bx = np.asarray(bx, dtype=np.float32)
    Wh = np.asarray(Wh, dtype=np.float32)

    xh = np.concatenate([x, h], axis=1)                      # [B, K]
    # [K, B] -> chunk-major [nchunk, 128 p, KO, BCHUNK] with k = ko*128+p.
    xh_sw = np.ascontiguousarray(
        xh.T.reshape(KO, 128, B // BCHUNK, BCHUNK).transpose(2, 1, 0, 3)
    )
    xh8_sw = _q8(xh_sw, SX)
    xhb_sw = np.ascontiguousarray(xh_sw[:, :, KT8:, :]).astype(BF16)
    xh00 = np.ascontiguousarray(
        xh.T.reshape(KO, 128, B)[:, :, :128].transpose(1, 0, 2)
    )
    xh00_8 = _q8(xh00, SX)

    Wcat = np.concatenate([Wx, Wh], axis=2)                  # [C, 2H, K]
    in_maps = []
    for c0 in range(NCORES):
        wc = np.concatenate(
            [Wcat[CPC * c0 + j].T for j in range(CPC)], axis=1
        )                                                    # [K, OPC]
        w_sl = wc.reshape(KO, 128, NSLAB, 512).transpose(2, 1, 0, 3)
        w8 = _q8(
            np.ascontiguousarray(w_sl[SIG_SLABS]), SW
        )                                                    # [4,128,KO,512]
        # tanh slabs: leading KT8 k-tiles in fp8 (laid out [128,4,KT8,512] so
        # the whole prefix loads as one 8KB-line descriptor), remainder in
        # bf16 pre-scaled by SCALE so both accumulate at a common PSUM scale.
        wt = w_sl[TANH_SLABS]                                # [4,128,KO,512]
        w8t = _q8(np.ascontiguousarray(wt[:, :, :KT8].transpose(1, 0, 2, 3)), SW)
        wb = np.ascontiguousarray(wt[:, :, KT8:] * SCALE).astype(BF16)
        bias_core = np.concatenate(
            [bx[CPC * c0 + j] for j in range(CPC)]
        ).astype(np.float32)                                 # [OPC]
        # every slab carries net*SCALE in PSUM; pre-scale the bias to match.
        bias_b = np.ascontiguousarray(
            np.broadcast_to((bias_core * SCALE).astype(BF16), (128, OPC))
        )
        in_maps.append(
            {
                "xh8": xh8_sw,
                "xhb": xhb_sw,
                "xh00": xh00_8,
                "w8": w8,
                "w8t": w8t,
                "wb": wb,
                "bias": bias_b,
            }
        )

    if "nc" not in _CACHE:
        _CACHE["nc"] = _build()
    nc = _CACHE["nc"]

    trace = bool(os.environ.get("GATE_TRACE"))
    if trace:
        _install_ntff_hook()
    res = bass_utils.run_bass_kernel_spmd(
        nc, in_maps, core_ids=list(range(NCORES)), trace=trace
    )
    _CACHE["last_result"] = res

    full = np.empty((B, C, 2 * H), np.float32)
    for c0 in range(NCORES):
        o = res.results[c0]["out"].astype(np.float32).reshape(B, CPC, 2 * H)
        for j in range(CPC):
            full[:, CPC * c0 + j, :] = o[:, j, :]
    input_gate = np.ascontiguousarray(full[:, :, :H])
    cell_input = np.ascontiguousarray(full[:, :, H:])
    return (cell_input, input_gate)


# revision 12
# speedup vs baseline: 1.1116x; 1.0032x over previous
"""Trainium2 Bass kernel for the stacked per-cell gate computation.

net[b,c,o] = sum_i x[b,i] Wx[c,o,i] + bx[c,o] + sum_h h[b,h] Wh[c,o,h]
cell_input = tanh(net[..., H:]);  input_gate = sigmoid(net[..., :H])

Strategy: concat x,h -> xh [B, 2048]; concat Wx,Wh per cell -> W' [2048 in,
2048 out].  Shard the C=16 cells as 2 per NeuronCore (expert parallel).  Each
core runs a [M=4096 b, K=2048, N=4096 o] matmul with a fused bias-add (DVE) +
sigmoid/tanh (ACT) epilogue, writing bf16.

Precision split: the sigmoid half of each cell's outputs is computed in
fp8-e4m3 with PE DoubleRow double-pumping (2x matmul throughput); the
sigmoid's flat transfer function absorbs the quantization error.  The tanh
half stays in bf16 (its steeper slope would push fp8 error past the accuracy
budget).

Phase schedule: sigmoid phases run one batch-chunk AHEAD of tanh phases
(S0 S1 T0 S2 T1 ... S7 T6 T7) so the bf16 tanh weights (8 MB) are not needed
until ~55us into the kernel, flattening the startup DMA demand curve.
"""

import os
from contextlib import ExitStack

import numpy as np
import ml_dtypes

B = 4096
IN = 1024
H = 1024
C = 16
NCORES = 8
CPC = C // NCORES          # cells per core
K = IN + H                 # contraction dim
KO = K // 128              # k-tiles
OPC = CPC * 2 * H          # output columns per core
NSLAB = OPC // 512         # 512-wide output slabs per core
BCHUNK = 512               # batch rows resident per xh chunk
NMC = B // BCHUNK

# slab n covers output cols [n*512, (n+1)*512); per cell: 2 sigmoid slabs
# then 2 tanh slabs.
SIG_SLABS = [0, 1, 4, 5]
TANH_SLABS = [2, 3, 6, 7]
W8_IDX = {0: 0, 1: 1, 4: 2, 5: 3}
WB_IDX = {2: 0, 3: 1, 6: 2, 7: 3}

SX = 16.0                  # xh fp8 pre-scale (power of 2: exact)
SW = 4096.0                # weight fp8 pre-scale
SCALE = SX * SW            # PSUM carries net*SCALE on every slab
# Leading k-tiles of the tanh half also run in fp8 DoubleRow; the bf16 tanh
# weights are pre-scaled by SCALE (exact power of 2) so both parts accumulate
# into one PSUM group at a common scale.  KT8=4 keeps rel err ~1.6e-2 (<2e-2).
KT8 = 4                    # tanh k-tiles (of KO=16) computed in fp8

BF16 = ml_dtypes.bfloat16
E4M3 = ml_dtypes.float8_e4m3   # IEEE e4m3 (bias 7, max 240) = TRN FP8_EXP4

_CACHE = {}


def _make_tc_class(tile, mybir, ScopedClock):
    """TileContext that never emits more than one sem-wait per instruction
    (this walrus build rejects multi-wait instructions in codegen)."""

    class SplitWaitTC(tile.TileContext):
        MAXW = 1

        def _split_waits(self, inst):
            si = getattr(inst, "sync_info", None)
            if si is None or len(si.on_wait) <= self.MAXW:
                return None
            waits = list(si.on_wait)
            inst.sync_info = mybir.SyncInfo(
                on_wait=waits[: self.MAXW], on_update=list(si.on_update)
            )
            nops = []
            for i in range(self.MAXW, len(waits), self.MAXW):
                nops.append(
                    mybir.InstNoOp(
                        name=self.nc.get_next_instruction_name(),
                        engine=inst.engine,
                        bass_nofuse=True,
                        sync_info=mybir.SyncInfo(
                            on_wait=waits[i : i + self.MAXW], on_update=[]
                        ),
                    )
                )
            return nops

        def _commit_and_lower(self, inst, original_block, old_bb_map, bb_to_exit_bb):
            nops = self._split_waits(inst)
            if nops:
                for nop in nops:
                    self._commit_instruction(nop)
            return super()._commit_and_lower(
                inst, original_block, old_bb_map, bb_to_exit_bb
            )

        def _drain_and_barrier(self, tick_clock, wait_clock):
            nc = self.nc
            drain_inst = nc.sync.drain()
            wait_clock.add_sem_waits(
                drain_inst.ins, ScopedClock({None: tick_clock.global_clock})
            )
            # Hoisting surplus waits onto trailing SP nops keeps semantics:
            # SP is FIFO, and the barrier below only passes once SP has
            # cleared every wait.
            si = drain_inst.ins.sync_info
            if si is not None and len(si.on_wait) > self.MAXW:
                waits = list(si.on_wait)
                drain_inst.ins.sync_info = mybir.SyncInfo(
                    on_wait=waits[: self.MAXW], on_update=list(si.on_update)
                )
                for i in range(self.MAXW, len(waits), self.MAXW):
                    nop = nc.sync.nop(nofuse=True)
                    nop.ins.sync_info = mybir.SyncInfo(
                        on_wait=waits[i : i + self.MAXW], on_update=[]
                    )
            nc.all_engine_barrier()
            assert self.sems is not None
            popped = nc._tile_sem_poison_stack.pop()
            assert popped is self._sem_poison
            nc.clear_and_free_semaphores(list(self.sems.allocated().values()))
            nc.all_engine_barrier()

    return SplitWaitTC


def _build():
    import concourse.bass as bass
    import concourse.tile as tile
    from concourse import mybir
    from concourse.vector_clock import ScopedClock

    SplitWaitTC = _make_tc_class(tile, mybir, ScopedClock)

    f32 = mybir.dt.float32
    bf16 = mybir.dt.bfloat16
    fp8 = mybir.dt.float8e4
    AF = mybir.ActivationFunctionType
    DR = mybir.MatmulPerfMode.DoubleRow

    nc = bass.Bass("TRN2", target_bir_lowering=False, debug=False)
    xh8_ap = nc.dram_tensor(
        "xh8", [NMC, 128, KO, BCHUNK], fp8, kind="ExternalInput"
    ).ap()
    xhb_ap = nc.dram_tensor(
        "xhb", [NMC, 128, KO - KT8, BCHUNK], bf16, kind="ExternalInput"
    ).ap()
    w8_ap = nc.dram_tensor(
        "w8", [len(SIG_SLABS), 128, KO, 512], fp8, kind="ExternalInput"
    ).ap()
    wb_ap = nc.dram_tensor(
        "wb", [len(TANH_SLABS), 128, KO - KT8, 512], bf16, kind="ExternalInput"
    ).ap()
    w8t_ap = nc.dram_tensor(
        "w8t", [128, len(TANH_SLABS), KT8, 512], fp8, kind="ExternalInput"
    ).ap()
    bias_ap = nc.dram_tensor("bias", [128, OPC], bf16, kind="ExternalInput").ap()
    out_ap = nc.dram_tensor("out", [B, OPC], bf16, kind="ExternalOutput").ap()

    with SplitWaitTC(nc) as tc:
        with ExitStack() as ctx:
            wpool = ctx.enter_context(tc.tile_pool(name="w", bufs=1))
            x8pool = ctx.enter_context(tc.tile_pool(name="xh8", bufs=3))
            xbpool = ctx.enter_context(tc.tile_pool(name="xhb", bufs=2))
            bpool = ctx.enter_context(tc.tile_pool(name="bias", bufs=1))
            pspool = ctx.enter_context(tc.tile_pool(name="ps", bufs=8, space="PSUM"))
            tpool = ctx.enter_context(tc.tile_pool(name="tmp", bufs=4))
            opool = ctx.enter_context(tc.tile_pool(name="o", bufs=6))

            w8_t = {}
            for n in SIG_SLABS:
                w8_t[n] = wpool.tile(
                    [128, KO, 512], fp8, tag=f"w8_{n}", name=f"w8_{n}"
                )
            wb_t = {}
            for n in TANH_SLABS:
                wb_t[n] = wpool.tile(
                    [128, KO - KT8, 512], bf16, tag=f"wb_{n}", name=f"wb_{n}"
                )
            w8t = wpool.tile(
                [128, len(TANH_SLABS), KT8, 512], fp8, tag="w8t", name="w8t"
            )
            xh8_sb = {}
            xhb_sb = {}
            xh8_sb[0] = x8pool.tile([128, KO, BCHUNK], fp8, tag="xh8", name="xh8_c0")
            xh8_sb[1] = x8pool.tile([128, KO, BCHUNK], fp8, tag="xh8", name="xh8_c1")
            xhb_sb[0] = xbpool.tile(
                [128, KO - KT8, BCHUNK], bf16, tag="xhb", name="xhb_c0"
            )
            bias_sb = bpool.tile([128, OPC], bf16)

            # Startup loads: one full-tensor descriptor each (>=4KB DMA lines
            # per partition; split descriptors tanked the queue to ~40GB/s),
            # in strict first-needed order per queue.  The sync queue carries
            # the critical path (w8 slabs + first fp8 chunk feed the opening
            # sigmoid phases); bf16 tanh data is not needed until ~60us
            # (phase T0), so it rides behind on the scalar/gpsimd queues.
            nc.sync.dma_start(w8_t[0][:], w8_ap[0, :, :, :])
            nc.scalar.dma_start(xh8_sb[0][:], xh8_ap[0, :, :, :])
            nc.sync.dma_start(w8_t[1][:], w8_ap[1, :, :, :])
            nc.scalar.dma_start(bias_sb[:], bias_ap[:])
            nc.sync.dma_start(w8_t[4][:], w8_ap[2, :, :, :])
            nc.sync.dma_start(w8_t[5][:], w8_ap[3, :, :, :])
            nc.scalar.dma_start(xhb_sb[0][:], xhb_ap[0, :, :, :])
            nc.scalar.dma_start(wb_t[2][:], wb_ap[0, :, :, :])
            nc.gpsimd.dma_start(xh8_sb[1][:], xh8_ap[1, :, :, :])
            nc.gpsimd.dma_start(w8t[:], w8t_ap[:])
            nc.gpsimd.dma_start(wb_t[3][:], wb_ap[1, :, :, :])
            nc.gpsimd.dma_start(wb_t[6][:], wb_ap[2, :, :, :])
            nc.gpsimd.dma_start(wb_t[7][:], wb_ap[3, :, :, :])

            out_rr = [0]

            def do_tile(mc, n, mi, nsplit, is_sig):
                func = AF.Sigmoid if is_sig else AF.Tanh
                width = 512 // nsplit
                row0 = mc * BCHUNK + mi * 128
                for sp in range(nsplit):
                    c0 = sp * width
                    ps = pspool.tile(
                        [128, width],
                        mybir.dt.float32,
                        tag="ps",
                        name=f"ps_{mc}_{n}_{mi}_{sp}",
                    )
                    if is_sig:
                        for k in range(KO // 2):
                            nc.tensor.matmul(
                                ps[:],
                                xh8_sb[mc][
                                    :, 2 * k : 2 * k + 2, mi * 128 : (mi + 1) * 128
                                ],
                                w8_t[n][:, 2 * k : 2 * k + 2, c0 : c0 + width],
                                start=(k == 0),
                                stop=(k == KO // 2 - 1),
                                perf_mode=DR,
                            )
                    else:
                        si = WB_IDX[n]
                        for k in range(KT8 // 2):
                            nc.tensor.matmul(
                                ps[:],
                                xh8_sb[mc][
                                    :, 2 * k : 2 * k + 2, mi * 128 : (mi + 1) * 128
                                ],
                                w8t[:, si, 2 * k : 2 * k + 2, c0 : c0 + width],
                                start=(k == 0),
                                stop=False,
                                perf_mode=DR,
                            )
                        for k in range(KO - KT8):
                            nc.tensor.matmul(
                                ps[:],
                                xhb_sb[mc][:, k, mi * 128 : (mi + 1) * 128],
                                wb_t[n][:, k, c0 : c0 + width],
                                start=False,
                                stop=(k == KO - KT8 - 1),
                            )
                    tmp = tpool.tile([128, width], mybir.dt.float32, tag="tmp")
                    nc.vector.tensor_tensor(
                        tmp[:],
                        ps[:],
                        bias_sb[:, n * 512 + c0 : n * 512 + c0 + width],
                        mybir.AluOpType.add,
                    )
                    o_t = opool.tile([128, width], bf16, tag="o")
                    nc.scalar.activation(o_t[:], tmp[:], func, scale=1.0 / SCALE)
                    out_q = (nc.sync, nc.scalar, nc.gpsimd)[out_rr[0] % 3]
                    out_rr[0] += 1
                    out_q.dma_start(
                        out_ap[
                            row0 : row0 + 128,
                            n * 512 + c0 : n * 512 + c0 + width,
                        ],
                        o_t[:],
                    )

            # Phase sequence: S0 S1 T0 S2 T1 ... S7 T6 T7.
            phases = [("S", 0)]
            for k in range(1, NMC):
                phases.append(("S", k))
                phases.append(("T", k - 1))
            phases.append(("T", NMC - 1))

            for kind, mc in phases:
                if kind == "S":
                    if mc >= 1:
                        # prefetch next chunks (~2 phases of lead time)
                        xhb_sb[mc] = xbpool.tile(
                            [128, KO - KT8, BCHUNK], bf16, tag="xhb",
                            name=f"xhb_c{mc}",
                        )
                        nc.sync.dma_start(xhb_sb[mc][:], xhb_ap[mc, :, :, :])
                        if mc + 1 < NMC and mc + 1 >= 2:
                            xh8_sb[mc + 1] = x8pool.tile(
                                [128, KO, BCHUNK], fp8, tag="xh8",
                                name=f"xh8_c{mc + 1}",
                            )
                            nc.gpsimd.dma_start(
                                xh8_sb[mc + 1][:], xh8_ap[mc + 1, :, :, :]
                            )
                    for n in SIG_SLABS:
                        for mi in range(BCHUNK // 128):
                            do_tile(mc, n, mi, 1, True)
                else:
                    last_phase = mc == NMC - 1
                    for n in TANH_SLABS:
                        for mi in range(BCHUNK // 128):
                            nsplit = (
                                4
                                if (last_phase and n == TANH_SLABS[-1]
                                    and mi == BCHUNK // 128 - 1)
                                else 1
                            )
                            do_tile(mc, n, mi, nsplit, False)
    return nc


def _q8(arr, scale):
    return np.clip(arr * scale, -240.0, 240.0).astype(E4M3)


def _install_ntff_hook():
    """Recreate the missing antenv.axon_hooks module so trace=True works."""
    import sys, types, ctypes, contextlib

    if "antenv.axon_hooks" in sys.modules:
        return
    so_path = "/opt/axon/libaxon_pjrt.so"
    lib = ctypes.CDLL(so_path)
    if not hasattr(lib, "axon_start_nrt_profile"):
        return
    lib.axon_start_nrt_profile.argtypes = [
        ctypes.POINTER(ctypes.c_int64),
        ctypes.c_size_t,
    ]
    lib.axon_start_nrt_profile.restype = ctypes.c_int64
    lib.axon_stop_nrt_profile.argtypes = [ctypes.c_char_p]
    lib.axon_stop_nrt_profile.restype = ctypes.c_int64

    @contextlib.contextmanager
    def _hook(output_dir, device_ids):
        import jax

        jax.devices()
        if device_ids:
            ids = (ctypes.c_int64 * len(device_ids))(*device_ids)
            rc = lib.axon_start_nrt_profile(ids, len(device_ids))
        else:
            rc = lib.axon_start_nrt_profile(None, 0)
        if rc != 0:
            raise RuntimeError(f"axon_start_nrt_profile rc={rc}")
        try:
            yield
        finally:
            n = lib.axon_stop_nrt_profile(str(output_dir).encode())
            if n < 0:
                raise RuntimeError(f"axon_stop_nrt_profile rc={n}")
            print(f"profile: {n} file(s) written to {output_dir}")

    mod = types.ModuleType("antenv.axon_hooks")
    mod.get_axon_ntff_profile_hook = lambda: _hook
    mod.set_axon_ntff_profile_hook = lambda h: None
    sys.modules["antenv.axon_hooks"] = mod


def kernel(input_word, hidden_states, Wx, bx, Wh):
    from concourse import bass_utils

    x = np.asarray(input_word, dtype=np.float32)
    h = np.asarray(hidden_states, dtype=np.float32)
    Wx = np.asarray(Wx, dtype=np.float32)
    bx = np.asarray(bx, dtype=np.float32)
    Wh = np.asarray(Wh, dtype=np.float32)

    xh = np.concatenate([x, h], axis=1)                      # [B, K]
    # [K, B] -> chunk-major [nchunk, 128 p, KO, BCHUNK] with k = ko*128+p.
    xh_sw = np.ascontiguousarray(
        xh.T.reshape(KO, 128, B // BCHUNK, BCHUNK).transpose(2, 1, 0, 3)
    )
    xh8_sw = _q8(xh_sw, SX)
    xhb_sw = np.ascontiguousarray(xh_sw[:, :, KT8:, :]).astype(BF16)

    Wcat = np.concatenate([Wx, Wh], axis=2)                  # [C, 2H, K]
    in_maps = []
    for c0 in range(NCORES):
        wc = np.concatenate(
            [Wcat[CPC * c0 + j].T for j in range(CPC)], axis=1
        )                                                    # [K, OPC]
        w_sl = wc.reshape(KO, 128, NSLAB, 512).transpose(2, 1, 0, 3)
        w8 = _q8(
            np.ascontiguousarray(w_sl[SIG_SLABS]), SW
        )                                                    # [4,128,KO,512]
        # tanh slabs: leading KT8 k-tiles in fp8 (laid out [128,4,KT8,512] so
        # the whole prefix loads as one 8KB-line descriptor), remainder in
        # bf16 pre-scaled by SCALE so both accumulate at a common PSUM scale.
        wt = w_sl[TANH_SLABS]                                # [4,128,KO,512]
        w8t = _q8(np.ascontiguousarray(wt[:, :, :KT8].transpose(1, 0, 2, 3)), SW)
        wb = np.ascontiguousarray(wt[:, :, KT8:] * SCALE).astype(BF16)
        bias_core = np.concatenate(
            [bx[CPC * c0 + j] for j in range(CPC)]
        ).astype(np.float32)                                 # [OPC]
        # every slab carries net*SCALE in PSUM; pre-scale the bias to match.
        bias_b = np.ascontiguousarray(
            np.broadcast_to((bias_core * SCALE).astype(BF16), (128, OPC))
        )
        in_maps.append(
            {
                "xh8": xh8_sw,
                "xhb": xhb_sw,
                "w8": w8,
                "w8t": w8t,
                "wb": wb,
                "bias": bias_b,
            }
        )

    if "nc" not in _CACHE:
        _CACHE["nc"] = _build()
    nc = _CACHE["nc"]

    trace = bool(os.environ.get("GATE_TRACE"))
    if trace:
        _install_ntff_hook()
    res = bass_utils.run_bass_kernel_spmd(
        nc, in_maps, core_ids=list(range(NCORES)), trace=trace
    )
    _CACHE["last_result"] = res

    full = np.empty((B, C, 2 * H), np.float32)
    for c0 in range(NCORES):
        o = res.results[c0]["out"].astype(np.float32).reshape(B, CPC, 2 * H)
        for j in range(CPC):
            full[:, CPC * c0 + j, :] = o[:, j, :]
    input_gate = np.ascontiguousarray(full[:, :, :H])
    cell_input = np.ascontiguousarray(full[:, :, H:])
    return (cell_input, input_gate)
